# revision 5
# baseline (speedup 1.0000x reference)
"""DialogueGCN forward on 8 Trainium2 NeuronCores.

Strategy (expert-parallel over relations, per the sharding hint):
  The dominant cost is streaming the per-edge relation weights
  rgcn_weight[etype].  The dense 10x10 edge list uses only U distinct
  relations out of R=200 (U=48 for the reference input), so we gather
  just the used slices.  By linearity the RGCN messages collapse to

      out = sum_u (A_u @ X) @ W_u       A_u = mean-normalized attention
                                         restricted to relation u

  one big contraction of [10, U*1024] x [U*1024, 1024].  The U*8
  128-row k-tiles are dealt evenly across the 8 cores (k-tile
  granularity -> no padding waste, perfect balance); each core
  accumulates its partial [10,1024] in PSUM and the partials are summed
  on the host along with the tiny root/bias and GraphConv epilogue.

  Device dtype is fp8 e4m3 with DoubleRow matmuls (256-row contraction
  per instruction): halves the HBM weight traffic vs bf16 -- the kernel
  is memory-bound -- and doubles PE throughput.  The fp8 quantization
  error of the weights dominates; H is carried as a hi+lo pair of fp8
  stationary columns at one common scale (out rows 0..9 = hi part,
  10..19 = lo part, folded on the host), which restores H to ~bf16
  accuracy for free.  PSUM is evicted in bf16 (fp16 would overflow).

  v2 perf structure (trace-driven):
  - PE column tiling: pair j is dealt to column group g = j % 3
    (tile_position (0, 32g) auto-derived from the PSUM slice
    acc[32g:32g+32]).  The three groups execute concurrently on
    disjoint PE column-group subarrays, so the PE consumes each DMA
    chunk ~3x faster than the single-group baseline and never becomes
    the pipeline tail -- even inside the HAM K=4/8 warmup window where
    PE utilization is capped at 50% (that warmup made the old PE end
    5 us after the stream and was the main cross-core variance).
  - Split eviction: DVE casts PSUM cols 0:512 -> SBUF bf16 and SP DMAs
    them out; ACT (the second HWDGE engine) casts cols 512:1024 and
    issues its own DMA.  The two 0.7us descriptor generations and the
    two copies run in parallel, shortening the post-matmul tail.
  - The weight stream itself already runs at the ~420 GB/s per-core
    SDMA cap (16 engines x ~26 GB/s) -- chunk schedule kept from v1.

  End-to-end L2 error vs the fp32 reference is ~6e-3 (threshold 2e-2).
"""

import os

import numpy as np

S = 10
N = 10
D = 1024
R = 2 * S * S
N_CORES = 8
KT = 128              # contraction rows per k-tile (partition dim)
KT_PER_REL = D // KT  # 8 k-tiles per relation
SH = 2.0 ** 7         # fp8 scale for H (hi and lo share it)
SW = 2.0 ** 11        # fp8 scale for W
F8_MAX = 440.0        # e4m3 saturation clip (max normal 448)

# Filled with the slowest profiled core's HW time (ns) when profiling is
# available (it is not under the axon PJRT path - stays None there).
LAST_EXEC_TIME_NS = None

_NC_CACHE = {}


def _ensure_ntff_hook():
    """Make traced runs survive environments without `antenv.axon_hooks`.

    Under axon, run_bass_kernel_spmd(trace=True) (or BASS_TRACE=1) does
    `from antenv.axon_hooks import get_axon_ntff_profile_hook` with no
    ImportError guard; images whose `antenv` package lacks that submodule
    crash.  Register the same ctypes NTFF hook the trn boot path would
    have installed, via the documented set/get interface.  No-op if the
    module already exists or the boot helpers are unavailable.
    """
    import sys
    import types

    if "antenv.axon_hooks" in sys.modules:
        return
    try:
        from trn_agent_boot.trn_boot import _ntff_profile_via_ctypes
    except ImportError:
        return
    try:
        hook = _ntff_profile_via_ctypes("/opt/axon/libaxon_pjrt.so")
    except OSError:
        return
    mod = types.ModuleType("antenv.axon_hooks")
    state = {"hook": hook}
    mod.set_axon_ntff_profile_hook = lambda h: state.__setitem__("hook", h)
    mod.get_axon_ntff_profile_hook = lambda: state["hook"]
    sys.modules["antenv.axon_hooks"] = mod
    try:
        import antenv

        antenv.axon_hooks = mod
    except ImportError:
        pass


def _groups():
    # PE column groups (1 = old single-group behaviour, 3 = col-tiling)
    return int(os.environ.get("BASS_GCN_GROUPS", "3"))


def _split_evict():
    return os.environ.get("BASS_GCN_SPLIT_EVICT", "1") == "1"


def _skip_osem():
    # Drop the trailing DMA-completion waits and rely on the NEFF
    # epilogue to drain the queues.  Faster but only enabled after
    # validation; default off.
    return os.environ.get("BASS_GCN_SKIP_OSEM", "0") == "1"


def _prog_tag():
    """Cache-busting tag baked into a tensor name: the PJRT compile cache
    keys on the HLO (which embeds the BIR), so walrus-flag experiments must
    perturb the BIR to take effect."""
    t = "t".join(str(x) for x in _taper())
    f = "f".join(str(x) for x in _front())
    return (
        f"se{int(_split_evict())}_tp{t}_fr{f}_b{_body()}"
        f"_g{_groups()}_o{int(_skip_osem())}_dr2"
    )


def _taper():
    t = os.environ.get("BASS_GCN_TAPER", "2,1,1")
    return [int(x) for x in t.split(",") if x]


def _body():
    # body chunk size in pairs; larger chunks keep the PE continuously
    # busy long enough (>3us) to reach the full 2.4GHz pstate
    return int(os.environ.get("BASS_GCN_BODY", "4"))


def _front():
    # empty by default: chunks smaller than ~4 pairs make the stream
    # descriptor-generation-bound (~0.7us SP desc-gen per chunk vs 0.69us
    # transfer per pair) and cost ~0.9us sem latency each
    t = os.environ.get("BASS_GCN_FRONT", "")
    return [int(x) for x in t.split(",") if x]


def _chunk_schedule(nkt2):
    """K-tile-pairs per DMA chunk.  Small chunks at the FRONT give the PE
    an early start; steady 4s in the body; tapering at the END so the
    PE/eviction tail after the last byte lands is minimal."""
    front, taper, bs = _front(), _taper(), _body()
    body = nkt2 - sum(taper) - sum(front)
    if body <= 0:
        return [1] * nkt2
    sched = [bs] * (body // bs)
    if body % bs:
        sched.append(body % bs)
    return front + sched + taper


def _build_nc(nkt2):
    """One core's program: out[OP,1024] = sum_j ht_j.T @ wt_j (DoubleRow).

    Raw bass (no TileContext), straight pipeline.  The weight stream is
    pre-tiled on the host into one flat [128, cols] region: the ht block
    (per-pair [2(pair), 32] stationary columns: 10 hi, 10 lo, 12 pad -
    DoubleRow stationary width must be a multiple of 16) first, then
    per-pair [2(bank), 2(pair), 512] weight slabs.  SP streams chunks on
    one HWDGE ring with a completion semaphore PER CHUNK (a cumulative
    counter is racy: later-chunk increments can stand in for missing
    earlier-chunk ones); PE chases with fp8e4 DoubleRow matmuls (256
    contraction rows each) into PSUM -- pair j lands in column group
    j % G at partitions [32g, 32g+32) so up to G matmuls run
    concurrently on disjoint PE column groups; DVE (and ACT when split
    eviction is on) evict to bf16; SP (and ACT) DMA the partials out;
    the host folds hi+lo across groups and rescales.
    """
    import contextlib

    import concourse.bass as bass
    import concourse.mybir as mybir

    G = _groups()
    assert nkt2 % G == 0
    sched = _chunk_schedule(nkt2)
    n_ch = len(sched)
    htc = nkt2 * 64                 # ht columns, prepended to chunk 0
    cols = htc + nkt2 * 2048
    f8 = mybir.dt.float8e4
    f32 = mybir.dt.float32
    DR = mybir.MatmulPerfMode.DoubleRow

    nc = bass.Bass()
    wt = nc.declare_dram_parameter("wt", [KT, cols], f8, isOutput=False)
    # 32 partitions per column group: hi rows 0..9, lo rows 10..19 (host
    # folds them - engines cannot add across partitions), rows 20..31
    # zero padding (DoubleRow stationary width must be a multiple of 16).
    OP = 32 * G
    bf16 = mybir.dt.bfloat16
    out = nc.declare_dram_parameter("out", [OP, D], bf16, isOutput=True)

    split = _split_evict()
    skip_osem = _skip_osem()
    # column ranges evicted+written by (SP/DVE, ACT)
    c_sp = (0, 512) if split else (0, 1024)

    with contextlib.ExitStack() as ctx:
        # one semaphore PER CHUNK: a single cumulative counter is unsound -
        # increments from a later chunk on the same queue can stand in for
        # missing increments from an earlier one, letting the PE consume a
        # chunk before it fully lands (cold-pass NaN race under profiling)
        wsem = [
            ctx.enter_context(nc.semaphore(f"wsem{c}")) for c in range(n_ch)
        ]
        osem = ctx.enter_context(nc.semaphore("osem"))
        psem = ctx.enter_context(nc.semaphore("psem"))
        vsem = ctx.enter_context(nc.semaphore("vsem"))
        osem2 = ctx.enter_context(nc.semaphore("osem2")) if split else None
        ws = ctx.enter_context(
            nc.sbuf_tensor(f"ws_{_prog_tag()}", [KT, cols], f8)
        )
        acc = ctx.enter_context(nc.psum_tensor("acc", [OP, D], f32))
        o_s = ctx.enter_context(nc.sbuf_tensor("os", [OP, D], bf16))
        # scratch for ACT's warmup copy (see below); never read back
        warm = (
            ctx.enter_context(nc.sbuf_tensor("warm", [1, 8], bf16))
            if split
            else None
        )

        # chunk c covers columns [starts[c] : starts[c+1])
        starts = [0]
        for i, ck in enumerate(sched):
            w = ck * 2048 + (htc if i == 0 else 0)
            starts.append(starts[-1] + w)

        with nc.Block() as block:

            @block.sync
            def _(sync):
                for c in range(n_ch):
                    sync.dma_start(
                        out=ws[:, starts[c] : starts[c + 1]],
                        in_=wt[:, starts[c] : starts[c + 1]],
                    ).then_inc(wsem[c], 16)
                sync.wait_ge(vsem, 2)
                sync.dma_start(
                    out=out[:, c_sp[0] : c_sp[1]],
                    in_=o_s[:, c_sp[0] : c_sp[1]],
                ).then_inc(osem, 16)
                if not skip_osem:
                    sync.wait_ge(osem, 16)

            @block.tensor
            def _(tensor):
                last_mm = None
                j = 0
                for c in range(n_ch):
                    tensor.wait_ge(wsem[c], 16)
                    for _g in range(sched[c]):
                        g = j % G
                        base = htc + j * 2048
                        first = j < G            # first pair of group g
                        last = j >= nkt2 - G     # last pair of group g
                        if G == 1:
                            # DoubleRow: 256 contraction rows per matmul.
                            # (The ISA rejects DoubleRow at a non-zero
                            # tile_position, so the col-tiled path below
                            # uses plain fp8 matmuls instead.)
                            lhsT = ws[:, j * 64 : (j + 1) * 64].rearrange(
                                "p (two f) -> p two f", two=2
                            )
                            tensor.matmul(
                                acc[0:32, 0:512],
                                lhsT,
                                ws[:, base : base + 1024].rearrange(
                                    "p (two f) -> p two f", two=2
                                ),
                                start=first, stop=last, perf_mode=DR,
                            )
                            last_mm = tensor.matmul(
                                acc[0:32, 512:1024],
                                lhsT,
                                ws[:, base + 1024 : base + 2048].rearrange(
                                    "p (two f) -> p two f", two=2
                                ),
                                start=first, stop=last, perf_mode=DR,
                            )
                        else:
                            # col-tiled plain fp8: per sub-k-tile p the
                            # stationary is [128, 32] and the two PSUM
                            # banks stream 512 cols each; group g's
                            # output lands at partitions [32g, 32g+32).
                            for p in range(2):
                                lhsT = ws[
                                    :, j * 64 + p * 32 : j * 64 + p * 32 + 32
                                ]
                                st = first and p == 0
                                sp = last and p == 1
                                for b in range(2):
                                    last_mm = tensor.matmul(
                                        acc[
                                            32 * g : 32 * g + 32,
                                            b * 512 : b * 512 + 512,
                                        ],
                                        lhsT,
                                        ws[
                                            :,
                                            base + b * 1024 + p * 512 :
                                            base + b * 1024 + p * 512 + 512,
                                        ],
                                        start=st, stop=sp,
                                    )
                        j += 1
                last_mm.then_inc(psem, 1)
                # drain the PE pipeline before signalling again: the
                # eviction must not observe PSUM before every accumulant
                # write commits (residual cold-pass NaN race, ~1/14 runs)
                tensor.drain()
                tensor.sem_inc(psem, 1)

            # PSUM eviction in bf16 (fp16 would overflow: |psum| ~ 1.3e5).
            @block.vector
            def _(vector):
                vector.wait_ge(psem, 2)
                vector.tensor_copy(
                    o_s[:, c_sp[0] : c_sp[1]], acc[:, c_sp[0] : c_sp[1]]
                ).then_inc(vsem, 1)
                # drain DVE so its SBUF writes are visible to the DMA
                # engines before the out transfer starts
                vector.drain()
                vector.sem_inc(vsem, 1)

            if split:
                # ACT evicts the other half and writes it out on its own
                # HWDGE ring - copy, descriptor generation and transfer
                # all run in parallel with the SP/DVE half.
                @block.scalar
                def _(scalar):
                    # warmup: the first InstActivation on ACT makes the
                    # compiler insert a ~1.3us ACT_TABLE_LOAD right before
                    # it.  Issue a throwaway 1-element copy while ACT is
                    # otherwise idle so the table load runs at program
                    # start instead of on the post-matmul critical path.
                    # `warm` is touched by nothing else, so this creates
                    # no dependency edges against the weight stream.
                    scalar.copy(warm[0:1, 0:1], warm[0:1, 4:5])
                    scalar.wait_ge(psem, 2)
                    scalar.copy(o_s[:, 512:1024], acc[:, 512:1024])
                    scalar.drain()
                    scalar.dma_start(
                        out=out[:, 512:1024], in_=o_s[:, 512:1024]
                    ).then_inc(osem2, 16)
                    if not skip_osem:
                        scalar.wait_ge(osem2, 16)

    return nc


def _q8(a, dt_np):
    return np.clip(a, -F8_MAX, F8_MAX).astype(dt_np)


def _prepare(global_features, speaker, Wq, Wk, rgcn_weight):
    """Host planning: attention, edge weights, per-relation aggregation,
    fp8 quantization, and the per-core slab arrays."""
    import ml_dtypes

    f8 = ml_dtypes.float8_e4m3

    x = np.asarray(global_features, dtype=np.float64)
    speaker = np.asarray(speaker)

    # ---- attention -> edge weights (tiny, host) ----
    q = x @ np.asarray(Wq, dtype=np.float64)
    k = x @ np.asarray(Wk, dtype=np.float64)
    logits = (q @ k.T) / np.sqrt(np.float64(D))
    logits -= logits.max(axis=-1, keepdims=True)
    attn = np.exp(logits)
    attn /= attn.sum(axis=-1, keepdims=True)

    # ---- dense all-pairs edges, relation ids, per-(rel,dst) mean weights ----
    src = np.repeat(np.arange(N), N)
    dst = np.tile(np.arange(N), N)
    sp = speaker.astype(np.int64)
    etype = 2 * (sp[src] * S + sp[dst]) + (src >= dst).astype(np.int64)
    used, inv = np.unique(etype, return_inverse=True)
    U = len(used)
    seg = etype * N + dst
    cnt = np.bincount(seg, minlength=R * N)
    w_e = attn.reshape(-1) / cnt[seg]

    # ---- per-used-relation aggregated pre-messages H[u,j,:] ----
    H = np.zeros((U, N, D))
    np.add.at(H, (inv, dst), w_e[:, None] * x[src])

    # ---- global k-tile lists, dealt in contiguous blocks to the cores ----
    G = _groups()
    nkt_tot = U * KT_PER_REL
    nkt = -(-nkt_tot // N_CORES)          # k-tiles per core
    nkt2 = -(-nkt // 2)                   # DoubleRow pairs per core
    nkt2 = -(-nkt2 // G) * G              # pad to a multiple of G pairs
    nkt = nkt2 * 2

    # H k-tiles [nkt_tot, 128, 10]; hi/lo fp8 split at one common scale
    Hkt = H.transpose(0, 2, 1).reshape(nkt_tot, KT, N)
    hi = _q8(Hkt * SH, f8)
    lo = _q8((Hkt - hi.astype(np.float64) / SH) * SH, f8)
    # W k-tiles [nkt_tot, 128, 1024] fp8
    Wkt = _q8(
        np.asarray(rgcn_weight)[used].reshape(nkt_tot, KT, D) * SW, f8
    )

    key = (nkt2, _prog_tag())
    if key not in _NC_CACHE:
        _NC_CACHE[key] = _build_nc(nkt2)
    nc = _NC_CACHE[key]

    htc = nkt2 * 64
    in_maps = []
    for c in range(N_CORES):
        lo_g = c * nkt
        n_real = min(nkt, max(0, nkt_tot - lo_g))

        h2 = np.zeros((nkt, KT, 32), dtype=f8)  # [kt, 128, hi|lo|pad cols]
        h2[:n_real, :, 0:N] = hi[lo_g : lo_g + n_real]
        h2[:n_real, :, N : 2 * N] = lo[lo_g : lo_g + n_real]
        w2 = np.zeros((nkt, KT, D), dtype=f8)
        w2[:n_real] = Wkt[lo_g : lo_g + n_real]

        flat = np.empty((KT, htc + nkt2 * 2048), dtype=f8)
        # ht block: cols j*64 + p*32 + m
        flat[:, :htc] = (
            h2.reshape(nkt2, 2, KT, 32)               # [j, p, 128, 32]
            .transpose(2, 0, 1, 3)                    # [128, j, p, 32]
            .reshape(KT, htc)
        )
        # weight block: cols htc + j*2048 + b*1024 + p*512 + col
        flat[:, htc:] = (
            w2.reshape(nkt2, 2, KT, 2, 512)           # [j, p, 128, b, 512]
            .transpose(2, 0, 3, 1, 4)                 # [128, j, b, p, 512]
            .reshape(KT, nkt2 * 2048)
        )
        in_maps.append({"wt": flat})

    return nc, in_maps, U


def kernel(global_features, speaker, Wq, Wk, rgcn_weight, rgcn_root,
           rgcn_bias, gcn_rel_w, gcn_rel_b, gcn_root_w):
    global LAST_EXEC_TIME_NS
    _ensure_ntff_hook()
    from concourse.bass_utils import run_bass_kernel_spmd

    nc, in_maps, _ = _prepare(global_features, speaker, Wq, Wk, rgcn_weight)

    kwargs = {}
    if os.environ.get("BASS_GCN_TRACE"):
        kwargs = dict(trace=True, trace_cores=list(range(N_CORES)))

    # The axon-tunneled device pool occasionally reports a transient
    # NRT_EXEC_UNIT_UNRECOVERABLE (unclean teardown of a *previous*
    # session); back off and retry before giving up.
    import time as _time

    last_err = None
    for attempt in range(3):
        try:
            res = run_bass_kernel_spmd(
                nc, in_maps, list(range(N_CORES)), **kwargs
            )
            break
        except Exception as e:  # noqa: BLE001
            last_err = e
            _time.sleep(5 * (attempt + 1))
    else:
        raise last_err
    LAST_EXEC_TIME_NS = res.exec_time_ns

    # ---- host all-reduce of the per-core partials (undo fp8 scales and
    # fold the hi/lo row blocks across column groups) ----
    G = _groups()
    out = np.zeros((N, D), dtype=np.float64)
    for r in res.results:
        p = r["out"].astype(np.float64)
        for g in range(G):
            out += p[32 * g : 32 * g + N] + p[32 * g + N : 32 * g + 2 * N]
    out /= SH * SW

    # ---- tiny epilogue on host ----
    x = np.asarray(global_features, dtype=np.float64)
    out += x @ np.asarray(rgcn_root, dtype=np.float64)
    out += np.asarray(rgcn_bias, dtype=np.float64)

    # GraphConv, sum aggregation over the dense edge list: every dst sees
    # every src, so agg is the column-sum of `out` broadcast to all rows.
    agg = np.broadcast_to(out.sum(axis=0), (N, D))
    x2 = (
        agg @ np.asarray(gcn_rel_w, dtype=np.float64)
        + np.asarray(gcn_rel_b, dtype=np.float64)
        + out @ np.asarray(gcn_root_w, dtype=np.float64)
    )

    return np.concatenate([x2, x], axis=-1).astype(np.float32)
